# revision 38
# baseline (speedup 1.0000x reference)
"""LSH bucketed sparse-attention kernel for Trainium2 (8 NeuronCores).

Strategy:
  - Host: conv embeddings + LSH hashing + stable counting-sort indices (cheap,
    data-dependent index prep) and final unsort/combine.
  - Device (8 cores, SPMD): the heavy bucketed attention. Core c handles hash
    h=c//2, chunk half v=c%2 (128 chunks of 144 sorted tokens each, with
    circular halo). Computes S^T = K-halfchunk-blocks vs Q windows, exp, and
    PV + denominator via matmul with a ones-column appended to V.
  - Math: the final cross-hash softmax combine == sum_h PV_h / sum_h denom_h,
    so the device returns unnormalized (PV, denom) and no per-hash
    normalization or logsumexp is needed anywhere.

Layout: Q and K are packed into 4 position-quarters at partition bases
0/32/64/96 (8 rows each) so the K=8-contraction matmuls of different quarters
run concurrently in distinct PE row-groups. Each Q quarter carries a one-chunk
halo on both sides so every S^T window stays inside its quarter.
"""

import numpy as np

N_HASHES = 4
CHUNK = 144
HC = 72  # half-chunk (key block unit)
L = 192 * 192  # 36864
L2 = L // 2  # per-core query positions (half of one hash's chunks)
EPS = 5e-5

QCOLS = 34 * CHUNK  # 32 chunks + 1 halo chunk each side = 4896
KHCS = 66  # max half-chunks per quarter (66/64/64/66, padded to 66)
HC_BASE = [0, 66, 130, 194]

_CACHED = {}


def _hc_quarter(hcl):
    lcp = hcl // 2 - 1  # local chunk of this half-chunk (-1..128 incl. halo)
    qq = min(max(lcp, 0), 127) // 32
    return qq, hcl - HC_BASE[qq]


def _window(lc):
    return [2 * (lcp + 1) + b for lcp in (lc - 1, lc, lc + 1) for b in (0, 1)]


def _st_args(hcl):
    """query window (clipped) + layout coords for half-chunk hcl."""
    lcp = hcl // 2 - 1
    lq0 = max(lcp - 1, 0)
    lq1 = min(lcp + 1, 127)
    n = (lq1 - lq0 + 1) * CHUNK
    qq, khc = _hc_quarter(hcl)
    qcol = (lq0 - 32 * qq + 1) * CHUNK
    return lq0, n, qq, khc, qcol


def _build_nc():
    import concourse.bass as bass
    import concourse.mybir as mybir

    F32 = mybir.dt.float32
    BF16 = mybir.dt.bfloat16
    Exp = mybir.ActivationFunctionType.Exp
    nc = bass.Bass()

    QK = QCOLS + KHCS * HC
    # compact: row 8*qq+e holds quarter qq's 8 embedding rows
    in_ext = nc.declare_dram_parameter("inp", [32, QK], BF16, isOutput=False)
    inv_ext = nc.declare_dram_parameter("inpv", [HC, 260 * 33], BF16, isOutput=False)
    out_ext = nc.declare_dram_parameter("out", [33, L2], BF16, isOutput=True)

    NPG = 64  # output groups of 2 chunks

    with (
        nc.sbuf_tensor([128, QK], BF16) as allsb,
        nc.sbuf_tensor([HC, 260 * 33], BF16) as vraw,
        nc.sbuf_tensor([128, 8, 512], BF16) as pts,  # P ring (exp scores)
        nc.sbuf_tensor([33, 2, 2 * CHUNK], BF16) as ots,  # out staging
        nc.psum_tensor([128, 4, 512], F32) as pst,   # S^T psum ring
        nc.psum_tensor([128, 2, 512], F32) as pot,   # PV psum ring
        nc.semaphore() as dma_sem,
        nc.semaphore() as st_sem,
        nc.semaphore() as exp_sem,
        nc.semaphore() as pv_sem,
        nc.semaphore() as cp_sem,
        nc.semaphore() as odma0,
        nc.semaphore() as odma1,
        nc.Block() as block,
    ):
        qsb = allsb[:, :QCOLS]
        ksb = allsb[:, QCOLS:]
        vsb = vraw[:].rearrange("p (a b) -> p a b", b=33)

        @block.sync
        def _(sync):
            for qq in range(4):
                sync.dma_start(
                    out=allsb[32 * qq : 32 * qq + 8, :],
                    in_=in_ext[8 * qq : 8 * qq + 8, :],
                ).then_inc(dma_sem, 16)
            sync.dma_start(out=vraw[:], in_=inv_ext[:]).then_inc(dma_sem, 16)
            for pg in range(NPG):
                sync.wait_ge(cp_sem, pg + 1)
                sync.dma_start(
                    out=out_ext[:, pg * 2 * CHUNK : (pg + 1) * 2 * CHUNK],
                    in_=ots[:, pg % 2, :],
                ).then_inc(odma0 if pg % 2 == 0 else odma1, 16)

        @block.tensor
        def _(tensor):
            tensor.wait_ge(dma_sem, 80)  # 4 Q/K quarter DMAs + V DMA
            for lc in range(128):
                # prefetch one chunk ahead so exp groups (4 half-chunks) can
                # always form without waiting on PV progress
                if lc == 0:
                    new = list(range(8))
                elif lc <= 126:
                    new = [2 * lc + 6, 2 * lc + 7]
                else:
                    new = []
                for hcl in new:
                    lq0, n, qq, khc, qcol = _st_args(hcl)
                    if hcl >= 4:
                        # psum slot free once the exp GROUP holding hcl-4 ran
                        tensor.wait_ge(exp_sem, (hcl - 4) // 4 + 1)
                    kw = {"tile_position": (96, 0)} if qq == 3 else {}
                    tensor.matmul(
                        pst[:HC, hcl % 4, :n],
                        lhsT=ksb[32 * qq : 32 * qq + 8, khc * HC : (khc + 1) * HC],
                        rhs=qsb[32 * qq : 32 * qq + 8, qcol : qcol + n],
                        start=True,
                        stop=True,
                        **kw,
                    ).then_inc(st_sem, 1)
                pg, i2 = lc // 2, lc % 2
                # exp groups (2 pairs each) through pair lc+2 done
                tensor.wait_ge(exp_sem, (lc + 2) // 2 + 1)
                if i2 == 0 and pg >= 2:
                    tensor.wait_ge(cp_sem, pg - 1)  # pot slot free
                wnd = _window(lc)
                for j, hcl in enumerate(wnd):
                    lq0 = _st_args(hcl)[0]
                    c0 = (lc - lq0) * CHUNK
                    mm = tensor.matmul(
                        pot[:33, pg % 2, i2 * CHUNK : (i2 + 1) * CHUNK],
                        lhsT=vsb[:, hcl, :],
                        rhs=pts[:HC, hcl % 8, c0 : c0 + CHUNK],
                        start=(j == 0),
                        stop=(j == 5),
                    )
                    if i2 == 1 and j == 5:
                        mm.then_inc(pv_sem, 1)

        @block.scalar
        def _(scalar):
            # one exp per GROUP of 2 chunks (4 half-chunks, the full psum
            # ring): slots are adjacent but windows differ in n, so use the
            # max n of the group (extra cols hold stale data, never read).
            for g in range(65):
                hcl = 4 * g
                scalar.wait_ge(st_sem, hcl + 4)
                if g >= 2:
                    # pt slots free once their last reader chunk's PV is done
                    last_rd = min(2 * (g - 2) + 1, 127)
                    scalar.wait_ge(pv_sem, last_rd // 2 + 1)
                if g == 0:
                    # pairs have different n and slots 0/1 are only partially
                    # written: split to avoid uninitialized psum reads
                    for pp in range(2):
                        n = _st_args(2 * pp)[1]
                        inst = scalar.activation(
                            pts[:HC, 2 * pp : 2 * pp + 2, :n],
                            pst[:HC, 2 * pp : 2 * pp + 2, :n],
                            Exp,
                        )
                        if pp == 1:
                            inst.then_inc(exp_sem, 1)
                else:
                    n = max(_st_args(hcl)[1], _st_args(hcl + 2)[1])
                    scalar.activation(
                        pts[:HC, hcl % 8 : hcl % 8 + 4, :n],
                        pst[:HC, 0:4, :n],
                        Exp,
                    ).then_inc(exp_sem, 1)

        @block.vector
        def _(vector):
            for pg in range(NPG):
                vector.wait_ge(pv_sem, pg + 1)
                if pg >= 2:
                    # out-dma of pg-2 (same slot parity) must be done
                    vector.wait_ge(odma0 if pg % 2 == 0 else odma1, 16 * (pg // 2))
                vector.tensor_copy(
                    out=ots[:, pg % 2, :], in_=pot[:33, pg % 2, : 2 * CHUNK]
                ).then_inc(cp_sem, 1)

    return nc


def _host_embed(x, w_match, b_match, w_asm, b_asm):
    xim = x[0]  # (32, 192, 192)
    xpad = np.pad(xim, ((0, 0), (1, 1), (1, 1)))
    xe = np.zeros((8, 192, 192), np.float32)
    for dy in range(3):
        for dx in range(3):
            xe += np.einsum(
                "oc,chw->ohw",
                w_match[:, :, dy, dx],
                xpad[:, dy : dy + 192, dx : dx + 192],
                optimize=True,
            )
    xe += b_match[:, None, None]
    x_embed = xe.reshape(8, L).T.astype(np.float32)  # (L, 8)
    ye = (
        np.einsum("oc,cl->ol", w_asm[:, :, 0, 0], xim.reshape(32, L), optimize=True)
        + b_asm[:, None]
    )
    y_embed = ye.T.astype(np.float32)  # (L, 32)
    return x_embed, y_embed


def kernel(x, w_match, b_match, w_asm, b_asm, rotations):
    from concourse.bass_utils import run_bass_kernel_spmd

    x = np.asarray(x, np.float32)
    w_match = np.asarray(w_match, np.float32)
    b_match = np.asarray(b_match, np.float32)
    w_asm = np.asarray(w_asm, np.float32)
    b_asm = np.asarray(b_asm, np.float32)
    rotations = np.asarray(rotations, np.float32)

    x_embed, y_embed = _host_embed(x, w_match, b_match, w_asm, b_asm)

    # LSH codes + stable sort (index prep only; attention math on device)
    rot = np.einsum("lf,fhi->hli", x_embed, rotations, optimize=True)  # (4, L, 64)
    # argmax over [rot, -rot] without materializing the 75MB concat:
    # first-max of rot vs first-min (= first max of -rot); ties (v1 == -v2)
    # resolve to the positive side, matching concat-argmax's first-occurrence
    i1 = rot.argmax(-1)
    i2 = rot.argmin(-1)
    v1 = np.take_along_axis(rot, i1[..., None], -1)[..., 0]
    v2 = np.take_along_axis(rot, i2[..., None], -1)[..., 0]
    codes = np.where(v1 >= -v2, i1, 64 + i2) + (np.arange(N_HASHES) * 128)[:, None]
    indices = np.argsort(codes.reshape(-1), kind="stable")
    undo = np.argsort(indices, kind="stable")
    mod = indices % L

    # gather + normalize once per hash (pair cores share the permutation)
    per_hash = []
    for h in range(N_HASHES):
        srt = mod[h * L : (h + 1) * L]
        ext = srt[np.arange(-CHUNK, L + CHUNK) % L]  # circular halo
        xg = x_embed[ext]  # (L+288, 8)
        yg = y_embed[ext]  # (L+288, 32)
        nrm = np.maximum(np.linalg.norm(xg, axis=-1, keepdims=True), EPS)
        per_hash.append((xg, xg / nrm, yg))

    in_maps = []
    for c in range(8):
        h, v = c // 2, c % 2
        xg, kg, yg = per_hash[h]
        s0 = v * L2  # ext index of position p0 - CHUNK
        qrows = xg[s0 + CHUNK : s0 + CHUNK + L2]  # (L2, 8) raw queries
        # Q quarters with 1-chunk halo each side; edge halos zero (never read)
        Qp = np.zeros((32, QCOLS), np.float32)
        qx = qrows.T  # (8, L2)
        for qq in range(4):
            s = qq * 32 * CHUNK
            lo, hi = s - CHUNK, s + 32 * CHUNK + CHUNK
            slo, shi = max(lo, 0), min(hi, L2)
            Qp[8 * qq : 8 * qq + 8, (slo - lo) : (slo - lo) + (shi - slo)] = qx[
                :, slo:shi
            ]
        Kh = kg[s0 : s0 + L2 + 2 * CHUNK].T.astype(np.float32)  # (8, 18720)
        Kr = Kh.reshape(8, 260, HC)
        Kp = np.zeros((32, KHCS * HC), np.float32)
        # hcl->quarter mapping is 4 contiguous ranges (66/64/64/66 blocks)
        bounds = HC_BASE + [260]
        for qq in range(4):
            b0, b1 = bounds[qq], bounds[qq + 1]
            Kp[8 * qq : 8 * qq + 8, : (b1 - b0) * HC] = Kr[:, b0:b1, :].reshape(8, -1)
        V = np.ones((18720, 33), np.float32)
        V[:, :32] = yg[s0 : s0 + L2 + 2 * CHUNK]
        Vp = V.reshape(260, HC, 33).transpose(1, 0, 2).reshape(HC, 260 * 33)
        import ml_dtypes

        in_maps.append(
            {
                "inp": np.ascontiguousarray(
                    np.concatenate([Qp, Kp], axis=1).astype(ml_dtypes.bfloat16)
                ),
                "inpv": np.ascontiguousarray(Vp.astype(ml_dtypes.bfloat16)),
            }
        )

    if "nc" not in _CACHED:
        _CACHED["nc"] = _build_nc()
    global _LAST_IN_MAPS
    _LAST_IN_MAPS = in_maps
    res = run_bass_kernel_spmd(_CACHED["nc"], in_maps, list(range(8)))

    # host combine: unsort + cross-hash merge (sum PV / sum denom)
    pv_sorted = np.zeros((N_HASHES * L, 32), np.float32)
    den_sorted = np.zeros((N_HASHES * L,), np.float32)
    for c in range(8):
        h, v = c // 2, c % 2
        o = np.asarray(res.results[c]["out"]).astype(np.float32)  # (33, L2)
        s = h * L + v * L2
        pv_sorted[s : s + L2] = o[:32].T
        den_sorted[s : s + L2] = o[32]
    pv = pv_sorted[undo].reshape(N_HASHES, L, 32)
    den = den_sorted[undo].reshape(N_HASHES, L)
    out_tok = pv.sum(0) / den.sum(0)[:, None]  # (L, 32)
    out = out_tok.T.reshape(1, 32, 192, 192) + x
    return out.astype(np.float32)


# revision 39
# speedup vs baseline: 1.1905x; 1.1905x over previous
"""LSH bucketed sparse-attention kernel for Trainium2 (8 NeuronCores).

Strategy:
  - Host: conv embeddings + LSH hashing + stable counting-sort indices (cheap,
    data-dependent index prep) and final unsort/combine.
  - Device (8 cores, SPMD): the heavy bucketed attention. Core c handles hash
    h=c//2, chunk half v=c%2 (128 chunks of 144 sorted tokens each, with
    circular halo). Computes S^T = K-halfchunk-blocks vs Q windows, exp, and
    PV + denominator via matmul with a ones-column appended to V.
  - Math: the final cross-hash softmax combine == sum_h PV_h / sum_h denom_h,
    so the device returns unnormalized (PV, denom) and no per-hash
    normalization or logsumexp is needed anywhere.

Layout: Q and K are packed into 4 position-quarters at partition bases
0/32/64/96 (8 rows each) so the K=8-contraction matmuls of different quarters
run concurrently in distinct PE row-groups. Each Q quarter carries a one-chunk
halo on both sides so every S^T window stays inside its quarter.
"""

import numpy as np

N_HASHES = 4
CHUNK = 144
HC = 72  # half-chunk (key block unit)
L = 192 * 192  # 36864
L2 = L // 2  # per-core query positions (half of one hash's chunks)
EPS = 5e-5

QCOLS = 34 * CHUNK  # 32 chunks + 1 halo chunk each side = 4896
KHCS = 66  # max half-chunks per quarter (66/64/64/66, padded to 66)
HC_BASE = [0, 66, 130, 194]

_CACHED = {}


def _hc_quarter(hcl):
    lcp = hcl // 2 - 1  # local chunk of this half-chunk (-1..128 incl. halo)
    qq = min(max(lcp, 0), 127) // 32
    return qq, hcl - HC_BASE[qq]


def _window(lc):
    return [2 * (lcp + 1) + b for lcp in (lc - 1, lc, lc + 1) for b in (0, 1)]


def _st_args(hcl):
    """query window (clipped) + layout coords for half-chunk hcl."""
    lcp = hcl // 2 - 1
    lq0 = max(lcp - 1, 0)
    lq1 = min(lcp + 1, 127)
    n = (lq1 - lq0 + 1) * CHUNK
    qq, khc = _hc_quarter(hcl)
    qcol = (lq0 - 32 * qq + 1) * CHUNK
    return lq0, n, qq, khc, qcol


def _build_nc():
    import concourse.bass as bass
    import concourse.mybir as mybir

    F32 = mybir.dt.float32
    BF16 = mybir.dt.bfloat16
    Exp = mybir.ActivationFunctionType.Exp
    nc = bass.Bass()

    QK = QCOLS + KHCS * HC
    # compact: row 8*qq+e holds quarter qq's 8 embedding rows
    in_ext = nc.declare_dram_parameter("inp", [32, QK], BF16, isOutput=False)
    inv_ext = nc.declare_dram_parameter("inpv", [HC, 260 * 33], BF16, isOutput=False)
    out_ext = nc.declare_dram_parameter("out", [33, L2], BF16, isOutput=True)

    NPG = 64  # output groups of 2 chunks

    with (
        nc.sbuf_tensor([128, QK], BF16) as allsb,
        nc.sbuf_tensor([HC, 260 * 33], BF16) as vraw,
        nc.sbuf_tensor([128, 8, 512], BF16) as pts,  # P ring (exp scores)
        nc.sbuf_tensor([33, 2, 2 * CHUNK], BF16) as ots,  # out staging
        nc.psum_tensor([128, 4, 512], F32) as pst,   # S^T psum ring
        nc.psum_tensor([128, 2, 512], F32) as pot,   # PV psum ring
        nc.semaphore() as dma_sem,
        nc.semaphore() as st_sem,
        nc.semaphore() as exp_sem,
        nc.semaphore() as pv_sem,
        nc.semaphore() as cp_sem,
        nc.semaphore() as odma0,
        nc.semaphore() as odma1,
        nc.Block() as block,
    ):
        qsb = allsb[:, :QCOLS]
        ksb = allsb[:, QCOLS:]
        vsb = vraw[:].rearrange("p (a b) -> p a b", b=33)

        @block.sync
        def _(sync):
            for qq in range(4):
                sync.dma_start(
                    out=allsb[32 * qq : 32 * qq + 8, :],
                    in_=in_ext[8 * qq : 8 * qq + 8, :],
                ).then_inc(dma_sem, 16)
            sync.dma_start(out=vraw[:], in_=inv_ext[:]).then_inc(dma_sem, 16)
            for pg in range(NPG):
                sync.wait_ge(cp_sem, pg + 1)
                sync.dma_start(
                    out=out_ext[:, pg * 2 * CHUNK : (pg + 1) * 2 * CHUNK],
                    in_=ots[:, pg % 2, :],
                ).then_inc(odma0 if pg % 2 == 0 else odma1, 16)

        @block.tensor
        def _(tensor):
            tensor.wait_ge(dma_sem, 80)  # 4 Q/K quarter DMAs + V DMA
            for lc in range(128):
                # prefetch one chunk ahead so exp groups (4 half-chunks) can
                # always form without waiting on PV progress
                if lc == 0:
                    new = list(range(8))
                elif lc <= 126:
                    new = [2 * lc + 6, 2 * lc + 7]
                else:
                    new = []
                for hcl in new:
                    lq0, n, qq, khc, qcol = _st_args(hcl)
                    if hcl >= 4:
                        # psum slot free once the exp GROUP holding hcl-4 ran
                        tensor.wait_ge(exp_sem, (hcl - 4) // 4 + 1)
                    kw = {"tile_position": (96, 0)} if qq == 3 else {}
                    tensor.matmul(
                        pst[:HC, hcl % 4, :n],
                        lhsT=ksb[32 * qq : 32 * qq + 8, khc * HC : (khc + 1) * HC],
                        rhs=qsb[32 * qq : 32 * qq + 8, qcol : qcol + n],
                        start=True,
                        stop=True,
                        **kw,
                    ).then_inc(st_sem, 1)
                pg, i2 = lc // 2, lc % 2
                # exp groups (2 pairs each) through pair lc+2 done
                tensor.wait_ge(exp_sem, (lc + 2) // 2 + 1)
                if i2 == 0 and pg >= 2:
                    tensor.wait_ge(cp_sem, pg - 1)  # pot slot free
                wnd = _window(lc)
                for j, hcl in enumerate(wnd):
                    lq0 = _st_args(hcl)[0]
                    c0 = (lc - lq0) * CHUNK
                    mm = tensor.matmul(
                        pot[:33, pg % 2, i2 * CHUNK : (i2 + 1) * CHUNK],
                        lhsT=vsb[:, hcl, :],
                        rhs=pts[:HC, hcl % 8, c0 : c0 + CHUNK],
                        start=(j == 0),
                        stop=(j == 5),
                    )
                    if i2 == 1 and j == 5:
                        mm.then_inc(pv_sem, 1)

        @block.scalar
        def _(scalar):
            # one exp per GROUP of 2 chunks (4 half-chunks, the full psum
            # ring): slots are adjacent but windows differ in n, so use the
            # max n of the group (extra cols hold stale data, never read).
            for g in range(65):
                hcl = 4 * g
                scalar.wait_ge(st_sem, hcl + 4)
                if g >= 2:
                    # pt slots free once their last reader chunk's PV is done
                    last_rd = min(2 * (g - 2) + 1, 127)
                    scalar.wait_ge(pv_sem, last_rd // 2 + 1)
                if g == 0:
                    # pairs have different n and slots 0/1 are only partially
                    # written: split to avoid uninitialized psum reads
                    for pp in range(2):
                        n = _st_args(2 * pp)[1]
                        inst = scalar.activation(
                            pts[:HC, 2 * pp : 2 * pp + 2, :n],
                            pst[:HC, 2 * pp : 2 * pp + 2, :n],
                            Exp,
                        )
                        if pp == 1:
                            inst.then_inc(exp_sem, 1)
                else:
                    n = max(_st_args(hcl)[1], _st_args(hcl + 2)[1])
                    scalar.activation(
                        pts[:HC, hcl % 8 : hcl % 8 + 4, :n],
                        pst[:HC, 0:4, :n],
                        Exp,
                    ).then_inc(exp_sem, 1)

        @block.vector
        def _(vector):
            for pg in range(NPG):
                vector.wait_ge(pv_sem, pg + 1)
                if pg >= 2:
                    # out-dma of pg-2 (same slot parity) must be done
                    vector.wait_ge(odma0 if pg % 2 == 0 else odma1, 16 * (pg // 2))
                vector.tensor_copy(
                    out=ots[:, pg % 2, :], in_=pot[:33, pg % 2, : 2 * CHUNK]
                ).then_inc(cp_sem, 1)

    return nc


def _host_embed(x, w_match, b_match, w_asm, b_asm):
    xim = x[0]  # (32, 192, 192)
    xpad = np.pad(xim, ((0, 0), (1, 1), (1, 1)))
    xe = np.zeros((8, 192, 192), np.float32)
    for dy in range(3):
        for dx in range(3):
            xe += np.einsum(
                "oc,chw->ohw",
                w_match[:, :, dy, dx],
                xpad[:, dy : dy + 192, dx : dx + 192],
                optimize=True,
            )
    xe += b_match[:, None, None]
    x_embed = xe.reshape(8, L).T.astype(np.float32)  # (L, 8)
    ye = (
        np.einsum("oc,cl->ol", w_asm[:, :, 0, 0], xim.reshape(32, L), optimize=True)
        + b_asm[:, None]
    )
    y_embed = ye.T.astype(np.float32)  # (L, 32)
    return x_embed, y_embed


def kernel(x, w_match, b_match, w_asm, b_asm, rotations):
    from concourse.bass_utils import run_bass_kernel_spmd

    x = np.asarray(x, np.float32)
    w_match = np.asarray(w_match, np.float32)
    b_match = np.asarray(b_match, np.float32)
    w_asm = np.asarray(w_asm, np.float32)
    b_asm = np.asarray(b_asm, np.float32)
    rotations = np.asarray(rotations, np.float32)

    x_embed, y_embed = _host_embed(x, w_match, b_match, w_asm, b_asm)

    # LSH codes + stable sort (index prep only; attention math on device)
    rot = np.einsum("lf,fhi->hli", x_embed, rotations, optimize=True)  # (4, L, 64)
    # argmax over [rot, -rot] without materializing the 75MB concat:
    # first-max of rot vs first-min (= first max of -rot); ties (v1 == -v2)
    # resolve to the positive side, matching concat-argmax's first-occurrence
    i1 = rot.argmax(-1)
    i2 = rot.argmin(-1)
    v1 = np.take_along_axis(rot, i1[..., None], -1)[..., 0]
    v2 = np.take_along_axis(rot, i2[..., None], -1)[..., 0]
    codes = np.where(v1 >= -v2, i1, 64 + i2) + (np.arange(N_HASHES) * 128)[:, None]
    indices = np.argsort(codes.reshape(-1), kind="stable")
    undo = np.argsort(indices, kind="stable")
    mod = indices % L

    # gather + normalize once per hash (pair cores share the permutation)
    per_hash = []
    for h in range(N_HASHES):
        srt = mod[h * L : (h + 1) * L]
        ext = srt[np.arange(-CHUNK, L + CHUNK) % L]  # circular halo
        xg = x_embed[ext]  # (L+288, 8)
        yg = y_embed[ext]  # (L+288, 32)
        nrm = np.maximum(np.linalg.norm(xg, axis=-1, keepdims=True), EPS)
        per_hash.append((xg, xg / nrm, yg))

    in_maps = []
    for c in range(8):
        h, v = c // 2, c % 2
        xg, kg, yg = per_hash[h]
        s0 = v * L2  # ext index of position p0 - CHUNK
        qrows = xg[s0 + CHUNK : s0 + CHUNK + L2]  # (L2, 8) raw queries
        # Q quarters with 1-chunk halo each side; edge halos zero (never read)
        Qp = np.zeros((32, QCOLS), np.float32)
        qx = qrows.T  # (8, L2)
        for qq in range(4):
            s = qq * 32 * CHUNK
            lo, hi = s - CHUNK, s + 32 * CHUNK + CHUNK
            slo, shi = max(lo, 0), min(hi, L2)
            Qp[8 * qq : 8 * qq + 8, (slo - lo) : (slo - lo) + (shi - slo)] = qx[
                :, slo:shi
            ]
        Kh = kg[s0 : s0 + L2 + 2 * CHUNK].T.astype(np.float32)  # (8, 18720)
        Kr = Kh.reshape(8, 260, HC)
        Kp = np.zeros((32, KHCS * HC), np.float32)
        # hcl->quarter mapping is 4 contiguous ranges (66/64/64/66 blocks)
        bounds = HC_BASE + [260]
        for qq in range(4):
            b0, b1 = bounds[qq], bounds[qq + 1]
            Kp[8 * qq : 8 * qq + 8, : (b1 - b0) * HC] = Kr[:, b0:b1, :].reshape(8, -1)
        V = np.ones((18720, 33), np.float32)
        V[:, :32] = yg[s0 : s0 + L2 + 2 * CHUNK]
        Vp = V.reshape(260, HC, 33).transpose(1, 0, 2).reshape(HC, 260 * 33)
        import ml_dtypes

        in_maps.append(
            {
                "inp": np.ascontiguousarray(
                    np.concatenate([Qp, Kp], axis=1).astype(ml_dtypes.bfloat16)
                ),
                "inpv": np.ascontiguousarray(Vp.astype(ml_dtypes.bfloat16)),
            }
        )

    if "nc" not in _CACHED:
        _CACHED["nc"] = _build_nc()
    global _LAST_IN_MAPS
    _LAST_IN_MAPS = in_maps
    res = run_bass_kernel_spmd(_CACHED["nc"], in_maps, list(range(8)))

    # host combine: unsort + cross-hash merge (sum PV / sum denom)
    pvden = np.zeros((N_HASHES * L, 33), np.float32)
    for c in range(8):
        h, v = c // 2, c % 2
        o = np.asarray(res.results[c]["out"]).astype(np.float32)  # (33, L2)
        pvden[h * L + v * L2 : h * L + (v + 1) * L2] = o.T
    tok = pvden[undo].reshape(N_HASHES, L, 33).sum(0)  # (L, 33)
    out_tok = tok[:, :32] / tok[:, 32:33]  # (L, 32)
    out = out_tok.T.reshape(1, 32, 192, 192) + x
    return out.astype(np.float32)
